# revision 22
# baseline (speedup 1.0000x reference)
"""Trainium2 Bass kernel for a pre-LN transformer block (attention + FFN).

x: [2, 2048, 1024] fp32, 16 heads, FFN hidden 4096.

Sharding: 8 cores = 2 batches x 4 token-quarters. Each core owns 512 query
tokens; K/V are computed redundantly for the full 2048-token batch on each
core (no collectives). All matmuls run in bf16 with fp32 PSUM accumulation.

Layout strategy (per core):
  - LayerNorm token-major [t, C] via bn_stats; rsqrt via ln+exp (one ACT set).
    LN scale/bias folded into weights/bias-rows on the host.
  - Activations transposed to feature-major [C, t] via DMA xbar transposes.
  - Q,K d-major [C, t]; V token-major [s, d] with an appended ones column so
    the attention-row sums fall out of the AV matmul (softmax without a
    separate reduction; no max-subtraction needed: |aff| <= ~3).
  - Attention: affT[s, t] = K_h.T @ Q_h (two heads packed per 128-partition
    tile, row-group concurrent), exp on ScalarE, OT[d, t] = V_ext.T @ expaff.
  - Normalization 1/rowsum broadcast along d via a tiny PE matmul (E matrix).
  - proj/FFN out token-major; residuals in fp32.
"""

import sys

sys.path.insert(0, "/opt/trn_rl_repo")

import numpy as np
import ml_dtypes

import concourse.bass as bass
import concourse.tile as tile
from concourse import bacc, mybir
from concourse import bass_utils

BF16 = mybir.dt.bfloat16
F32 = mybir.dt.float32
AF = mybir.ActivationFunctionType
OP = mybir.AluOpType

N_CORES = 8
B, T, C = 2, 2048, 1024
H, D = 16, 64
F = 4 * C
TOWN = T // 4  # 512 own query tokens per core
LN_EPS = 1e-5

_CACHED_NC = None


def _body(tc):
    nc = tc.nc
    x_own = nc.dram_tensor("x_own", [TOWN, C], F32, kind="ExternalInput").ap()
    x_kv = nc.dram_tensor("x_kv", [T, C], F32, kind="ExternalInput").ap()
    wq_d = nc.dram_tensor("wq", [8, 8, 128, 128], BF16, kind="ExternalInput").ap()
    wk_d = nc.dram_tensor("wk", [8, 8, 128, 128], BF16, kind="ExternalInput").ap()
    wv_d = nc.dram_tensor("wv", [8, 128, 1024], BF16, kind="ExternalInput").ap()
    wp_d = nc.dram_tensor("wp", [8, 128, 1024], BF16, kind="ExternalInput").ap()
    w1_d = nc.dram_tensor("w1", [32, 8, 128, 128], BF16, kind="ExternalInput").ap()
    w2_d = nc.dram_tensor("w2", [32, 128, 1024], BF16, kind="ExternalInput").ap()
    bcolq_d = nc.dram_tensor("bcolq", [128, 8], F32, kind="ExternalInput").ap()
    bcolk_d = nc.dram_tensor("bcolk", [128, 8], F32, kind="ExternalInput").ap()
    b1col_d = nc.dram_tensor("b1col", [128, 32], F32, kind="ExternalInput").ap()
    bv_d = nc.dram_tensor("bv", [1, 1024], BF16, kind="ExternalInput").ap()
    bp_d = nc.dram_tensor("bp_r", [1, 1024], BF16, kind="ExternalInput").ap()
    b2_d = nc.dram_tensor("b2_r", [1, 1024], BF16, kind="ExternalInput").ap()
    out_d = nc.dram_tensor("out", [TOWN, C], F32, kind="ExternalOutput").ap()

    big = tc.alloc_tile_pool(name="big", bufs=1)
    xres = tc.alloc_tile_pool(name="xres", bufs=1)

    K_sb = big.tile([128, 8, 2048], BF16, name="K_sb")
    V_sb = big.tile([128, 16, 16, 66], BF16, name="V_sb")
    QT_sb = big.tile([128, 8, 512], BF16, name="QT_sb")
    OT_un = big.tile([128, 8, 512], BF16, name="OT_un")
    OT_n = big.tile([128, 8, 512], BF16, name="OT_n")
    bcolq_sb = big.tile([128, 8], F32, name="bcolq_sb")
    bcolk_sb = big.tile([128, 8], F32, name="bcolk_sb")
    b1col_sb = big.tile([128, 32], F32, name="b1col_sb")
    bv_sb = big.tile([1, 1024], BF16, name="bv_sb")
    bp_sb = big.tile([1, 1024], BF16, name="bp_sb")
    b2_sb = big.tile([1, 1024], BF16, name="b2_sb")
    ones_sb = big.tile([1, 512], BF16, name="ones_sb")
    eps_sb = big.tile([128, 1], F32, name="eps_sb")
    nc.vector.memset(eps_sb[:], LN_EPS)

    nc.sync.dma_start(bcolq_sb[:], bcolq_d[:])
    nc.sync.dma_start(bcolk_sb[:], bcolk_d[:])
    nc.sync.dma_start(b1col_sb[:], b1col_d[:])
    nc.sync.dma_start(bv_sb[:], bv_d[:])
    nc.sync.dma_start(bp_sb[:], bp_d[:])
    nc.sync.dma_start(b2_sb[:], b2_d[:])
    nc.vector.memset(ones_sb[:], 1.0)
    nc.vector.memset(V_sb[:, :, :, 64:65], 1.0)

    # x_own tiles (also used for residual), x2 tiles, out tiles share slots
    x_own_t = []
    for i in range(4):
        xo = xres.tile([128, 1024], F32, tag="xbig", bufs=8, name=f"x_own_{i}")
        nc.sync.dma_start(xo[:], x_own[i * 128:(i + 1) * 128, :])
        x_own_t.append(xo)

    def layernorm_tile(pool, src_ap, name):
        """src_ap: [128, 1024] fp32 (SBUF or freshly DMA'd). Returns hn bf16."""
        st6 = pool.tile([128, 12], F32, tag="st6", bufs=3, name=f"st6_{name}")
        nc.vector.bn_stats(st6[:, 0:6], src_ap[:, 0:512])
        nc.vector.bn_stats(st6[:, 6:12], src_ap[:, 512:1024])
        ag = pool.tile([128, 2], F32, tag="ag", bufs=3, name=f"ag_{name}")
        nc.vector.bn_aggr(ag[:], st6[:])
        lnv = pool.tile([128, 1], F32, tag="lnv", bufs=3, name=f"lnv_{name}")
        nc.scalar.activation(lnv[:], ag[:, 1:2], AF.Ln, bias=eps_sb[:])
        rsig = pool.tile([128, 1], F32, tag="rsig", bufs=3, name=f"rsig_{name}")
        nc.scalar.activation(rsig[:], lnv[:], AF.Exp, scale=-0.5)
        hn = pool.tile([128, 1024], BF16, tag="hn", bufs=3, name=f"hn_{name}")
        nc.vector.tensor_scalar(hn[:], src_ap, ag[:, 0:1], rsig[:],
                                op0=OP.subtract, op1=OP.mult)
        return hn

    # ---- LN1 over KV tokens + QKV projections, in two token halves ----
    for half in range(2):
        with tc.tile_pool(name=f"qkvh{half}", bufs=1) as hp_pool, \
             tc.tile_pool(name=f"qkvw{half}", bufs=1) as wpool, \
             tc.tile_pool(name=f"qkvp{half}", bufs=4, space="PSUM") as qk_psum:
            hT = hp_pool.tile([128, 8, 1024], BF16, name=f"hT_{half}")
            for i8 in range(8):
                i = 8 * half + i8
                xt = hp_pool.tile([128, 1024], F32, tag="xkv", bufs=3,
                                  name=f"xkv_{i}")
                nc.sync.dma_start(xt[:], x_kv[i * 128:(i + 1) * 128, :])
                hn = layernorm_tile(hp_pool, xt[:], f"kv{i}")
                for cj in range(8):
                    nc.sync.dma_start_transpose(
                        hT[:, cj, i8 * 128:(i8 + 1) * 128],
                        hn[:, cj * 128:(cj + 1) * 128])
            # K projection: K_sb[:, dt, tb] (d-major), columns of this half
            for dt in range(8):
                wts = []
                for kt in range(8):
                    wt = wpool.tile([128, 128], BF16, tag="wkq", bufs=18,
                                    name=f"wk_{half}_{dt}_{kt}")
                    nc.sync.dma_start(wt[:], wk_d[dt, kt])
                    wts.append(wt)
                for tb2 in range(2):
                    ps = qk_psum.tile([128, 512], F32, tag="qkvps",
                                      name=f"psK_{half}_{dt}_{tb2}")
                    for kt in range(8):
                        nc.tensor.matmul(ps[:], wts[kt][:],
                                         hT[:, kt, tb2 * 512:(tb2 + 1) * 512],
                                         start=(kt == 0), stop=(kt == 7))
                    tb = 2 * half + tb2
                    nc.vector.tensor_scalar(
                        K_sb[:, dt, tb * 512:(tb + 1) * 512], ps[:],
                        bcolk_sb[:, dt:dt + 1], None, op0=OP.add)
            # V projection: token-major with head-interleaved layout
            wvt = []
            for kt in range(8):
                wv = wpool.tile([128, 1024], BF16, tag="wv", bufs=8,
                                name=f"wv_{half}_{kt}")
                nc.sync.dma_start(wv[:], wv_d[kt])
                wvt.append(wv)
            for tt8 in range(8):
                tt = 8 * half + tt8
                for db in range(2):
                    ps = qk_psum.tile([128, 512], F32, tag="qkvps",
                                      name=f"psV_{tt}_{db}")
                    for kt in range(8):
                        nc.tensor.matmul(ps[:],
                                         hT[:, kt, tt8 * 128:(tt8 + 1) * 128],
                                         wvt[kt][:, db * 512:(db + 1) * 512],
                                         start=(kt == 0), stop=False)
                    nc.tensor.matmul(ps[:], ones_sb[:, 0:128],
                                     bv_sb[:, db * 512:(db + 1) * 512],
                                     start=False, stop=True)
                    nc.vector.tensor_copy(
                        V_sb[:, tt, db * 8:(db + 1) * 8, 0:64],
                        ps.rearrange("p (h d) -> p h d", d=64))

    # ---- LN1 over own tokens + Q projection ----
    with tc.tile_pool(name="qown", bufs=1) as qo_pool, \
         tc.tile_pool(name="qoww", bufs=1) as wpool, \
         tc.tile_pool(name="qop", bufs=4, space="PSUM") as q_psum:
        hTo = qo_pool.tile([128, 8, 512], BF16, name="hTo")
        for i in range(4):
            hn = layernorm_tile(qo_pool, x_own_t[i][:], f"own{i}")
            for cj in range(8):
                nc.sync.dma_start_transpose(
                    hTo[:, cj, i * 128:(i + 1) * 128],
                    hn[:, cj * 128:(cj + 1) * 128])
        for dt in range(8):
            wts = []
            for kt in range(8):
                wt = wpool.tile([128, 128], BF16, tag="wq", bufs=18,
                                name=f"wq_{dt}_{kt}")
                nc.sync.dma_start(wt[:], wq_d[dt, kt])
                wts.append(wt)
            ps = q_psum.tile([128, 512], F32, tag="qps", name=f"psQ_{dt}")
            for kt in range(8):
                nc.tensor.matmul(ps[:], wts[kt][:], hTo[:, kt, :],
                                 start=(kt == 0), stop=(kt == 7))
            nc.vector.tensor_scalar(QT_sb[:, dt, :], ps[:],
                                    bcolq_sb[:, dt:dt + 1], None, op0=OP.add)

    # ---- attention + softmax normalization ----
    with tc.tile_pool(name="anorm", bufs=1) as an_pool:
        sumflat = an_pool.tile([1, 16 * 512], F32, name="sumflat")
        rcpb = an_pool.tile([1, 16 * 512], BF16, name="rcpb")
        with tc.tile_pool(name="attn", bufs=1) as at_pool, \
             tc.tile_pool(name="affp", bufs=2, space="PSUM") as aff_psum, \
             tc.tile_pool(name="otp", bufs=4, space="PSUM") as ot_psum:
            for hp in range(8):
                otA = ot_psum.tile([65, 512], F32, tag="ot", name=f"otA_{hp}")
                otB = ot_psum.tile([65, 512], F32, tag="ot", name=f"otB_{hp}")
                for st in range(16):
                    aff = aff_psum.tile([128, 1024], F32, tag="aff",
                                        name=f"aff_{hp}_{st}")
                    nc.tensor.matmul(aff[:, 0:512],
                                     K_sb[0:64, hp, st * 128:(st + 1) * 128],
                                     QT_sb[0:64, hp, :], start=True, stop=True)
                    nc.tensor.matmul(aff[:, 512:1024],
                                     K_sb[64:128, hp, st * 128:(st + 1) * 128],
                                     QT_sb[64:128, hp, :], start=True,
                                     stop=True)
                    ex = at_pool.tile([128, 1024], BF16, tag="ex", bufs=3,
                                      name=f"ex_{hp}_{st}")
                    nc.scalar.activation(ex[:], aff[:], AF.Exp, scale=0.125)
                    nc.tensor.matmul(otA[:], V_sb[:, st, 2 * hp, 0:65],
                                     ex[:, 0:512], start=(st == 0),
                                     stop=(st == 15))
                    nc.tensor.matmul(otB[:], V_sb[:, st, 2 * hp + 1, 0:65],
                                     ex[:, 512:1024], start=(st == 0),
                                     stop=(st == 15))
                nc.vector.tensor_copy(OT_un[0:64, hp, :], otA[0:64, :])
                nc.vector.tensor_copy(OT_un[64:128, hp, :], otB[0:64, :])
                hA, hB = 2 * hp, 2 * hp + 1
                nc.vector.tensor_copy(sumflat[:, hA * 512:(hA + 1) * 512],
                                      otA[64:65, :])
                nc.vector.tensor_copy(sumflat[:, hB * 512:(hB + 1) * 512],
                                      otB[64:65, :])
        with tc.tile_pool(name="nrm", bufs=1) as n_pool, \
             tc.tile_pool(name="nrmp", bufs=4, space="PSUM") as rb_psum:
            for h in range(16):
                rt = n_pool.tile([1, 512], F32, tag="rt", bufs=2,
                                 name=f"rt_{h}")
                nc.vector.reciprocal(rt[:], sumflat[:, h * 512:(h + 1) * 512])
                nc.vector.tensor_copy(rcpb[:, h * 512:(h + 1) * 512], rt[:])
            for hp in range(8):
                rbp = rb_psum.tile([128, 512], F32, tag="rbp",
                                   name=f"rbp_{hp}")
                hA, hB = 2 * hp, 2 * hp + 1
                nc.tensor.matmul(rbp[0:64, :], ones_sb[:, 0:64],
                                 rcpb[:, hA * 512:(hA + 1) * 512],
                                 start=True, stop=True)
                nc.tensor.matmul(rbp[64:128, :], ones_sb[:, 0:64],
                                 rcpb[:, hB * 512:(hB + 1) * 512],
                                 start=True, stop=True)
                rb = n_pool.tile([128, 512], BF16, tag="rb", bufs=3,
                                 name=f"rb_{hp}")
                nc.vector.tensor_copy(rb[:], rbp[:])
                nc.vector.tensor_mul(OT_n[:, hp, :], OT_un[:, hp, :], rb[:])

    # ---- proj + residual ----
    x2_t = []
    with tc.tile_pool(name="proj", bufs=1) as pj_pool, \
         tc.tile_pool(name="projp", bufs=4, space="PSUM") as pj_psum:
        wpt = []
        for hp in range(8):
            wp = pj_pool.tile([128, 1024], BF16, tag="wp", bufs=8,
                              name=f"wp_{hp}")
            nc.sync.dma_start(wp[:], wp_d[hp])
            wpt.append(wp)
        for tt in range(4):
            x2 = xres.tile([128, 1024], F32, tag="xbig", bufs=8,
                           name=f"x2_{tt}")
            for cb in range(2):
                ps = pj_psum.tile([128, 512], F32, tag="pjps",
                                  name=f"psP_{tt}_{cb}")
                for hp in range(8):
                    nc.tensor.matmul(ps[:],
                                     OT_n[:, hp, tt * 128:(tt + 1) * 128],
                                     wpt[hp][:, cb * 512:(cb + 1) * 512],
                                     start=(hp == 0), stop=False)
                nc.tensor.matmul(ps[:], ones_sb[:, 0:128],
                                 bp_sb[:, cb * 512:(cb + 1) * 512],
                                 start=False, stop=True)
                nc.vector.tensor_add(x2[:, cb * 512:(cb + 1) * 512], ps[:],
                                     x_own_t[tt][:, cb * 512:(cb + 1) * 512])
            x2_t.append(x2)

    # ---- LN2 + FFN ----
    with tc.tile_pool(name="ffn", bufs=1) as f_pool, \
         tc.tile_pool(name="ffnw", bufs=1) as fw_pool, \
         tc.tile_pool(name="ffnp", bufs=4, space="PSUM") as f_psum:
        hT2 = f_pool.tile([128, 8, 512], BF16, name="hT2")
        g1T = f_pool.tile([128, 32, 512], BF16, name="g1T")
        for i in range(4):
            hn = layernorm_tile(f_pool, x2_t[i][:], f"ln2_{i}")
            for cj in range(8):
                nc.sync.dma_start_transpose(
                    hT2[:, cj, i * 128:(i + 1) * 128],
                    hn[:, cj * 128:(cj + 1) * 128])
        for ft in range(32):
            ps = f_psum.tile([128, 512], F32, tag="fps", name=f"psF_{ft}")
            for kt in range(8):
                wt = fw_pool.tile([128, 128], BF16, tag="w1", bufs=18,
                                  name=f"w1_{ft}_{kt}")
                nc.sync.dma_start(wt[:], w1_d[ft, kt])
                nc.tensor.matmul(ps[:], wt[:], hT2[:, kt, :],
                                 start=(kt == 0), stop=(kt == 7))
            nc.scalar.activation(g1T[:, ft, :], ps[:], AF.Gelu,
                                 bias=b1col_sb[:, ft:ft + 1])

    with tc.tile_pool(name="ffo", bufs=1) as fo_pool, \
         tc.tile_pool(name="ffop", bufs=1, space="PSUM") as fo_psum:
        fo = [fo_psum.tile([128, 512], F32, tag=f"fo{i}", name=f"fo_{i}")
              for i in range(8)]
        for ft in range(32):
            w2t = fo_pool.tile([128, 1024], BF16, tag="w2", bufs=4,
                               name=f"w2_{ft}")
            nc.sync.dma_start(w2t[:], w2_d[ft])
            for tt in range(4):
                for cb in range(2):
                    nc.tensor.matmul(fo[2 * tt + cb][:],
                                     g1T[:, ft, tt * 128:(tt + 1) * 128],
                                     w2t[:, cb * 512:(cb + 1) * 512],
                                     start=(ft == 0), stop=False)
        for tt in range(4):
            o = xres.tile([128, 1024], F32, tag="xbig", bufs=8,
                          name=f"out_sb_{tt}")
            for cb in range(2):
                nc.tensor.matmul(fo[2 * tt + cb][:], ones_sb[:, 0:128],
                                 b2_sb[:, cb * 512:(cb + 1) * 512],
                                 start=False, stop=True)
                nc.vector.tensor_add(o[:, cb * 512:(cb + 1) * 512],
                                     fo[2 * tt + cb][:],
                                     x2_t[tt][:, cb * 512:(cb + 1) * 512])
            nc.sync.dma_start(out_d[tt * 128:(tt + 1) * 128, :], o[:])

    xres.release()
    big.release()


def build_nc():
    nc = bacc.Bacc("TRN2", target_bir_lowering=False, debug=False,
                   num_devices=N_CORES)
    with tile.TileContext(nc) as tc:
        _body(tc)
    nc.compile()
    return nc


def _prep_weights(Wq, Wk, Wv, Wp, bp, W1, b1, W2, b2, g1, be1, g2, be2):
    bf = ml_dtypes.bfloat16
    g1 = g1.astype(np.float32)
    g2 = g2.astype(np.float32)

    def fold(W, g):
        return (g[:, None] * W.astype(np.float32))

    Wq_f, Wk_f, Wv_f = fold(Wq, g1), fold(Wk, g1), fold(Wv, g1)
    W1_f = fold(W1, g2)
    bq = be1.astype(np.float32) @ Wq.astype(np.float32)
    bk = be1.astype(np.float32) @ Wk.astype(np.float32)
    bv = be1.astype(np.float32) @ Wv.astype(np.float32)
    b1f = be2.astype(np.float32) @ W1.astype(np.float32) + b1.astype(np.float32)

    def tile_dt_kt(W):  # [C, C] -> [8 dt, 8 kt, 128, 128]
        return np.ascontiguousarray(
            W.reshape(8, 128, 8, 128).transpose(2, 0, 1, 3)).astype(bf)

    wq_t = tile_dt_kt(Wq_f)
    wk_t = tile_dt_kt(Wk_f)
    wv_t = np.ascontiguousarray(Wv_f.reshape(8, 128, 1024)).astype(bf)
    wp_t = np.ascontiguousarray(
        Wp.astype(np.float32).reshape(8, 128, 1024)).astype(bf)
    w1_t = np.ascontiguousarray(
        W1_f.reshape(8, 128, 32, 128).transpose(2, 0, 1, 3)).astype(bf)
    w2_t = np.ascontiguousarray(
        W2.astype(np.float32).reshape(32, 128, 1024)).astype(bf)
    bcolq = np.ascontiguousarray(bq.reshape(8, 128).T).astype(np.float32)
    bcolk = np.ascontiguousarray(bk.reshape(8, 128).T).astype(np.float32)
    b1col = np.ascontiguousarray(b1f.reshape(32, 128).T).astype(np.float32)
    return dict(wq=wq_t, wk=wk_t, wv=wv_t, wp=wp_t, w1=w1_t, w2=w2_t,
                bcolq=bcolq, bcolk=bcolk, b1col=b1col,
                bv=bv.reshape(1, 1024).astype(bf),
                bp_r=bp.astype(np.float32).reshape(1, 1024).astype(bf),
                b2_r=b2.astype(np.float32).reshape(1, 1024).astype(bf))


def kernel(x, Wq, Wk, Wv, Wp, bp, W1, b1, W2, b2, g1, be1, g2, be2):
    global _CACHED_NC
    x = np.asarray(x, dtype=np.float32)
    if _CACHED_NC is None:
        _CACHED_NC = build_nc()
    nc = _CACHED_NC
    w = _prep_weights(np.asarray(Wq), np.asarray(Wk), np.asarray(Wv),
                      np.asarray(Wp), np.asarray(bp), np.asarray(W1),
                      np.asarray(b1), np.asarray(W2), np.asarray(b2),
                      np.asarray(g1), np.asarray(be1), np.asarray(g2),
                      np.asarray(be2))
    in_maps = []
    for c in range(N_CORES):
        b, q = c // 4, c % 4
        m = dict(w)
        m["x_own"] = np.ascontiguousarray(x[b, q * TOWN:(q + 1) * TOWN, :])
        m["x_kv"] = np.ascontiguousarray(x[b])
        in_maps.append(m)
    res = bass_utils.run_bass_kernel_spmd(nc, in_maps,
                                          core_ids=list(range(N_CORES)))
    out = np.empty((B, T, C), dtype=np.float32)
    for c in range(N_CORES):
        b, q = c // 4, c % 4
        out[b, q * TOWN:(q + 1) * TOWN, :] = res.results[c]["out"]
    return out


# revision 33
# speedup vs baseline: 6.8501x; 6.8501x over previous
"""Trainium2 Bass kernel for a pre-LN transformer block (attention + FFN).

x: [2, 2048, 1024] fp32, 16 heads, FFN hidden 4096.

Sharding: 8 cores = 2 batches x 4 token-quarters. Each core owns 512 query
tokens; K/V are computed redundantly for the full 2048-token batch on each
core (no collectives). All matmuls run in bf16 with fp32 PSUM accumulation.

Layout strategy (per core):
  - LayerNorm token-major [t, C] via bn_stats; rsqrt via ln+exp (one ACT set).
    LN scale/bias folded into weights/bias-rows on the host.
  - Activations transposed to feature-major [C, t] via DMA xbar transposes.
  - Q,K d-major [C, t]; V token-major [s, d] with an appended ones column so
    the attention-row sums fall out of the AV matmul (softmax without a
    separate reduction; no max-subtraction needed: |aff| <= ~3).
  - Attention: affT[s, t] = K_h.T @ Q_h (two heads packed per 128-partition
    tile, row-group concurrent), exp on ScalarE, OT[d, t] = V_ext.T @ expaff.
  - Normalization 1/rowsum broadcast along d via a tiny PE matmul (E matrix).
  - proj/FFN out token-major; residuals in fp32.
"""

import sys

sys.path.insert(0, "/opt/trn_rl_repo")

import numpy as np
import ml_dtypes

import concourse.bass as bass
import concourse.tile as tile
from concourse import bacc, mybir
from concourse import bass_utils

BF16 = mybir.dt.bfloat16
F32 = mybir.dt.float32
AF = mybir.ActivationFunctionType
OP = mybir.AluOpType

N_CORES = 8
B, T, C = 2, 2048, 1024
H, D = 16, 64
F = 4 * C
TOWN = T // 4  # 512 own query tokens per core
LN_EPS = 1e-5

_CACHED_NC = None


def _body(tc):
    nc = tc.nc
    x_own = nc.dram_tensor("x_own", [TOWN, C], F32, kind="ExternalInput").ap()
    x_kv = nc.dram_tensor("x_kv", [T, C], F32, kind="ExternalInput").ap()
    wq_d = nc.dram_tensor("wq", [8, 128, 8, 128], BF16, kind="ExternalInput").ap()
    wk_d = nc.dram_tensor("wk", [8, 128, 8, 128], BF16, kind="ExternalInput").ap()
    wv_d = nc.dram_tensor("wv", [8, 128, 1024], BF16, kind="ExternalInput").ap()
    wp_d = nc.dram_tensor("wp", [8, 128, 1024], BF16, kind="ExternalInput").ap()
    w1_d = nc.dram_tensor("w1", [32, 128, 8, 128], BF16, kind="ExternalInput").ap()
    w2_d = nc.dram_tensor("w2", [32, 128, 1024], BF16, kind="ExternalInput").ap()
    id_d = nc.dram_tensor("ident", [128, 128], BF16, kind="ExternalInput").ap()
    bcolq_d = nc.dram_tensor("bcolq", [128, 8], F32, kind="ExternalInput").ap()
    bcolk_d = nc.dram_tensor("bcolk", [128, 8], F32, kind="ExternalInput").ap()
    b1col_d = nc.dram_tensor("b1col", [128, 32], F32, kind="ExternalInput").ap()
    bv_d = nc.dram_tensor("bv", [1, 1024], BF16, kind="ExternalInput").ap()
    bp_d = nc.dram_tensor("bp_r", [1, 1024], BF16, kind="ExternalInput").ap()
    b2_d = nc.dram_tensor("b2_r", [1, 1024], BF16, kind="ExternalInput").ap()
    out_d = nc.dram_tensor("out", [TOWN, C], F32, kind="ExternalOutput").ap()

    big = tc.alloc_tile_pool(name="big", bufs=1)
    xres = tc.alloc_tile_pool(name="xres", bufs=1)

    K_sb = big.tile([128, 8, 2048], BF16, name="K_sb")
    V_sb = big.tile([128, 16, 16, 66], BF16, name="V_sb")
    QT_sb = big.tile([128, 8, 512], BF16, name="QT_sb")
    OT_un = big.tile([128, 8, 512], BF16, name="OT_un")
    OT_n = big.tile([128, 8, 512], BF16, name="OT_n")
    bcolq_sb = big.tile([128, 8], F32, name="bcolq_sb")
    bcolk_sb = big.tile([128, 8], F32, name="bcolk_sb")
    b1col_sb = big.tile([128, 32], F32, name="b1col_sb")
    bv_sb = big.tile([1, 1024], BF16, name="bv_sb")
    bp_sb = big.tile([1, 1024], BF16, name="bp_sb")
    b2_sb = big.tile([1, 1024], BF16, name="b2_sb")
    ones_sb = big.tile([1, 512], BF16, name="ones_sb")
    ident_sb = big.tile([128, 128], BF16, name="ident_sb")
    nc.sync.dma_start(ident_sb[:], id_d[:])
    eps_sb = big.tile([128, 1], F32, name="eps_sb")
    nc.vector.memset(eps_sb[:], LN_EPS)

    nc.sync.dma_start(bcolq_sb[:], bcolq_d[:])
    nc.sync.dma_start(bcolk_sb[:], bcolk_d[:])
    nc.sync.dma_start(b1col_sb[:], b1col_d[:])
    nc.sync.dma_start(bv_sb[:], bv_d[:])
    nc.sync.dma_start(bp_sb[:], bp_d[:])
    nc.sync.dma_start(b2_sb[:], b2_d[:])
    nc.vector.memset(ones_sb[:], 1.0)
    nc.vector.memset(V_sb[:, :, :, 64:65], 1.0)

    # x_own tiles (also used for residual), x2 tiles, out tiles share slots
    x_own_t = []
    for i in range(4):
        xo = xres.tile([128, 1024], F32, tag="xbig", bufs=8, name=f"x_own_{i}")
        nc.sync.dma_start(xo[:], x_own[i * 128:(i + 1) * 128, :])
        x_own_t.append(xo)

    def layernorm_tile(pool, src_ap, name):
        """src_ap: [128, 1024] fp32 (SBUF or freshly DMA'd). Returns hn bf16."""
        st6 = pool.tile([128, 12], F32, tag="st6", bufs=3, name=f"st6_{name}")
        nc.vector.bn_stats(st6[:, 0:6], src_ap[:, 0:512])
        nc.vector.bn_stats(st6[:, 6:12], src_ap[:, 512:1024])
        ag = pool.tile([128, 2], F32, tag="ag", bufs=3, name=f"ag_{name}")
        nc.vector.bn_aggr(ag[:], st6[:])
        lnv = pool.tile([128, 1], F32, tag="lnv", bufs=3, name=f"lnv_{name}")
        nc.scalar.activation(lnv[:], ag[:, 1:2], AF.Ln, bias=eps_sb[:])
        rsig = pool.tile([128, 1], F32, tag="rsig", bufs=3, name=f"rsig_{name}")
        nc.scalar.activation(rsig[:], lnv[:], AF.Exp, scale=-0.5)
        hn = pool.tile([128, 1024], BF16, tag="hn", bufs=3, name=f"hn_{name}")
        nc.vector.tensor_scalar(hn[:], src_ap, ag[:, 0:1], rsig[:],
                                op0=OP.subtract, op1=OP.mult)
        return hn

    def transpose_waves(tp_psum, hT, hn, iw, nw, tag, state):
        """PE-transpose hn [128,1024] into hT[:, cj, iw*128:...]; bf16 PSUM
        accumulates the whole section (nw blocks <= one bank), one evac."""
        if iw == 0:
            state["tp"] = [tp_psum.tile([128, nw * 128], BF16, tag=f"tp{cj}",
                                        name=f"tp_{tag}_{cj}")
                           for cj in range(8)]
        for cj in range(8):
            tp = state["tp"][cj]
            nc.tensor.transpose(tp[:, iw * 128:(iw + 1) * 128],
                                hn[:, cj * 128:(cj + 1) * 128], ident_sb[:])
            if iw == nw - 1:
                nc.vector.tensor_copy(hT[:, cj, :], tp[:])

    # ---- LN1 over KV tokens + QKV projections, in two token halves ----
    for half in range(2):
        with tc.tile_pool(name=f"qkvh{half}", bufs=1) as hp_pool, \
             tc.tile_pool(name=f"qkvw{half}", bufs=1) as wpool:
            hT = hp_pool.tile([128, 8, 1024], BF16, name=f"hT_{half}")
            with tc.tile_pool(name=f"tp{half}", bufs=1,
                              space="PSUM") as tp_psum:
                tps = {}
                for i8 in range(8):
                    i = 8 * half + i8
                    xt = hp_pool.tile([128, 1024], F32, tag="xkv", bufs=3,
                                      name=f"xkv_{i}")
                    nc.sync.dma_start(xt[:], x_kv[i * 128:(i + 1) * 128, :])
                    hn = layernorm_tile(hp_pool, xt[:], f"kv{i}")
                    transpose_waves(tp_psum, hT, hn, i8, 8, f"kv{half}", tps)
            with tc.tile_pool(name=f"qkvp{half}", bufs=4,
                              space="PSUM") as qk_psum:
                # K projection: K_sb[:, dt, tb] (d-major)
                for dt in range(8):
                    wkq = wpool.tile([128, 8, 128], BF16, tag="wkq", bufs=4,
                                     name=f"wk_{half}_{dt}")
                    nc.sync.dma_start(wkq[:], wk_d[dt])
                    for tb2 in range(2):
                        ps = qk_psum.tile([128, 512], F32, tag="qkvps",
                                          name=f"psK_{half}_{dt}_{tb2}")
                        for kt in range(8):
                            nc.tensor.matmul(
                                ps[:], wkq[:, kt, :],
                                hT[:, kt, tb2 * 512:(tb2 + 1) * 512],
                                start=(kt == 0), stop=(kt == 7))
                        tb = 2 * half + tb2
                        nc.vector.tensor_scalar(
                            K_sb[:, dt, tb * 512:(tb + 1) * 512], ps[:],
                            bcolk_sb[:, dt:dt + 1], None, op0=OP.add)
                # V projection: token-major with head-interleaved layout
                wvt = []
                for kt in range(8):
                    wv = wpool.tile([128, 1024], BF16, tag="wv", bufs=8,
                                    name=f"wv_{half}_{kt}")
                    nc.sync.dma_start(wv[:], wv_d[kt])
                    wvt.append(wv)
                for tt8 in range(8):
                    tt = 8 * half + tt8
                    for db in range(2):
                        ps = qk_psum.tile([128, 512], F32, tag="qkvps",
                                          name=f"psV_{tt}_{db}")
                        for kt in range(8):
                            nc.tensor.matmul(
                                ps[:],
                                hT[:, kt, tt8 * 128:(tt8 + 1) * 128],
                                wvt[kt][:, db * 512:(db + 1) * 512],
                                start=(kt == 0), stop=False)
                        nc.tensor.matmul(ps[:], ones_sb[:, 0:128],
                                         bv_sb[:, db * 512:(db + 1) * 512],
                                         start=False, stop=True)
                        nc.vector.tensor_copy(
                            V_sb[:, tt, db * 8:(db + 1) * 8, 0:64],
                            ps.rearrange("p (h d) -> p h d", d=64))

    # ---- LN1 over own tokens + Q projection ----
    with tc.tile_pool(name="qown", bufs=1) as qo_pool, \
         tc.tile_pool(name="qoww", bufs=1) as wpool:
        hTo = qo_pool.tile([128, 8, 512], BF16, name="hTo")
        with tc.tile_pool(name="tpo", bufs=1, space="PSUM") as tp_psum:
            tps = {}
            for i in range(4):
                hn = layernorm_tile(qo_pool, x_own_t[i][:], f"own{i}")
                transpose_waves(tp_psum, hTo, hn, i, 4, "own", tps)
        with tc.tile_pool(name="qop", bufs=4, space="PSUM") as q_psum:
            for dt in range(8):
                wq = wpool.tile([128, 8, 128], BF16, tag="wq", bufs=4,
                                name=f"wq_{dt}")
                nc.sync.dma_start(wq[:], wq_d[dt])
                ps = q_psum.tile([128, 512], F32, tag="qps", name=f"psQ_{dt}")
                for kt in range(8):
                    nc.tensor.matmul(ps[:], wq[:, kt, :], hTo[:, kt, :],
                                     start=(kt == 0), stop=(kt == 7))
                nc.vector.tensor_scalar(QT_sb[:, dt, :], ps[:],
                                        bcolq_sb[:, dt:dt + 1], None,
                                        op0=OP.add)

    # ---- attention + softmax normalization ----
    with tc.tile_pool(name="anorm", bufs=1) as an_pool:
        sumflat = an_pool.tile([1, 16 * 512], F32, name="sumflat")
        rcpb = an_pool.tile([1, 16 * 512], BF16, name="rcpb")
        with tc.tile_pool(name="attn", bufs=1) as at_pool, \
             tc.tile_pool(name="affp", bufs=2, space="PSUM") as aff_psum, \
             tc.tile_pool(name="otp", bufs=4, space="PSUM") as ot_psum:
            for hp in range(8):
                otA = ot_psum.tile([65, 512], F32, tag="ot", name=f"otA_{hp}")
                otB = ot_psum.tile([65, 512], F32, tag="ot", name=f"otB_{hp}")
                for st in range(16):
                    aff = aff_psum.tile([128, 1024], F32, tag="aff",
                                        name=f"aff_{hp}_{st}")
                    nc.tensor.matmul(aff[:, 0:512],
                                     K_sb[0:64, hp, st * 128:(st + 1) * 128],
                                     QT_sb[0:64, hp, :], start=True, stop=True)
                    nc.tensor.matmul(aff[:, 512:1024],
                                     K_sb[64:128, hp, st * 128:(st + 1) * 128],
                                     QT_sb[64:128, hp, :], start=True,
                                     stop=True)
                    ex = at_pool.tile([128, 1024], BF16, tag="ex", bufs=3,
                                      name=f"ex_{hp}_{st}")
                    nc.scalar.activation(ex[:], aff[:], AF.Exp, scale=0.125)
                    nc.tensor.matmul(otA[:], V_sb[:, st, 2 * hp, 0:65],
                                     ex[:, 0:512], start=(st == 0),
                                     stop=(st == 15))
                    nc.tensor.matmul(otB[:], V_sb[:, st, 2 * hp + 1, 0:65],
                                     ex[:, 512:1024], start=(st == 0),
                                     stop=(st == 15))
                nc.vector.tensor_copy(OT_un[0:64, hp, :], otA[0:64, :])
                nc.vector.tensor_copy(OT_un[64:128, hp, :], otB[0:64, :])
                hA, hB = 2 * hp, 2 * hp + 1
                nc.vector.tensor_copy(sumflat[:, hA * 512:(hA + 1) * 512],
                                      otA[64:65, :])
                nc.vector.tensor_copy(sumflat[:, hB * 512:(hB + 1) * 512],
                                      otB[64:65, :])
        with tc.tile_pool(name="nrm", bufs=1) as n_pool, \
             tc.tile_pool(name="nrmp", bufs=4, space="PSUM") as rb_psum:
            for h in range(16):
                rt = n_pool.tile([1, 512], F32, tag="rt", bufs=2,
                                 name=f"rt_{h}")
                nc.vector.reciprocal(rt[:], sumflat[:, h * 512:(h + 1) * 512])
                nc.vector.tensor_copy(rcpb[:, h * 512:(h + 1) * 512], rt[:])
            for hp in range(8):
                rbp = rb_psum.tile([128, 512], F32, tag="rbp",
                                   name=f"rbp_{hp}")
                hA, hB = 2 * hp, 2 * hp + 1
                nc.tensor.matmul(rbp[0:64, :], ones_sb[:, 0:64],
                                 rcpb[:, hA * 512:(hA + 1) * 512],
                                 start=True, stop=True)
                nc.tensor.matmul(rbp[64:128, :], ones_sb[:, 0:64],
                                 rcpb[:, hB * 512:(hB + 1) * 512],
                                 start=True, stop=True)
                rb = n_pool.tile([128, 512], BF16, tag="rb", bufs=3,
                                 name=f"rb_{hp}")
                nc.vector.tensor_copy(rb[:], rbp[:])
                nc.vector.tensor_mul(OT_n[:, hp, :], OT_un[:, hp, :], rb[:])

    # ---- proj + residual ----
    x2_t = []
    with tc.tile_pool(name="proj", bufs=1) as pj_pool, \
         tc.tile_pool(name="projp", bufs=4, space="PSUM") as pj_psum:
        wpt = []
        for hp in range(8):
            wp = pj_pool.tile([128, 1024], BF16, tag="wp", bufs=8,
                              name=f"wp_{hp}")
            nc.sync.dma_start(wp[:], wp_d[hp])
            wpt.append(wp)
        for tt in range(4):
            x2 = xres.tile([128, 1024], F32, tag="xbig", bufs=8,
                           name=f"x2_{tt}")
            for cb in range(2):
                ps = pj_psum.tile([128, 512], F32, tag="pjps",
                                  name=f"psP_{tt}_{cb}")
                for hp in range(8):
                    nc.tensor.matmul(ps[:],
                                     OT_n[:, hp, tt * 128:(tt + 1) * 128],
                                     wpt[hp][:, cb * 512:(cb + 1) * 512],
                                     start=(hp == 0), stop=False)
                nc.tensor.matmul(ps[:], ones_sb[:, 0:128],
                                 bp_sb[:, cb * 512:(cb + 1) * 512],
                                 start=False, stop=True)
                nc.vector.tensor_add(x2[:, cb * 512:(cb + 1) * 512], ps[:],
                                     x_own_t[tt][:, cb * 512:(cb + 1) * 512])
            x2_t.append(x2)

    # ---- LN2 + FFN ----
    with tc.tile_pool(name="ffn", bufs=1) as f_pool, \
         tc.tile_pool(name="ffnw", bufs=1) as fw_pool:
        hT2 = f_pool.tile([128, 8, 512], BF16, name="hT2")
        g1T = f_pool.tile([128, 32, 512], BF16, name="g1T")
        with tc.tile_pool(name="tp2", bufs=1, space="PSUM") as tp_psum:
            tps = {}
            for i in range(4):
                hn = layernorm_tile(f_pool, x2_t[i][:], f"ln2_{i}")
                transpose_waves(tp_psum, hT2, hn, i, 4, "ln2", tps)
        with tc.tile_pool(name="ffnp", bufs=4, space="PSUM") as f_psum:
            for ft in range(32):
                w1t = fw_pool.tile([128, 8, 128], BF16, tag="w1", bufs=4,
                                   name=f"w1_{ft}")
                nc.sync.dma_start(w1t[:], w1_d[ft])
                ps = f_psum.tile([128, 512], F32, tag="fps", name=f"psF_{ft}")
                for kt in range(8):
                    nc.tensor.matmul(ps[:], w1t[:, kt, :], hT2[:, kt, :],
                                     start=(kt == 0), stop=(kt == 7))
                nc.scalar.activation(g1T[:, ft, :], ps[:], AF.Gelu,
                                     bias=b1col_sb[:, ft:ft + 1])

    with tc.tile_pool(name="ffo", bufs=1) as fo_pool, \
         tc.tile_pool(name="ffop", bufs=1, space="PSUM") as fo_psum:
        fo = [fo_psum.tile([128, 512], F32, tag=f"fo{i}", name=f"fo_{i}")
              for i in range(8)]
        for ft in range(32):
            w2t = fo_pool.tile([128, 1024], BF16, tag="w2", bufs=4,
                               name=f"w2_{ft}")
            nc.sync.dma_start(w2t[:], w2_d[ft])
            for tt in range(4):
                for cb in range(2):
                    nc.tensor.matmul(fo[2 * tt + cb][:],
                                     g1T[:, ft, tt * 128:(tt + 1) * 128],
                                     w2t[:, cb * 512:(cb + 1) * 512],
                                     start=(ft == 0), stop=False)
        for tt in range(4):
            o = xres.tile([128, 1024], F32, tag="xbig", bufs=8,
                          name=f"out_sb_{tt}")
            for cb in range(2):
                nc.tensor.matmul(fo[2 * tt + cb][:], ones_sb[:, 0:128],
                                 b2_sb[:, cb * 512:(cb + 1) * 512],
                                 start=False, stop=True)
                nc.vector.tensor_add(o[:, cb * 512:(cb + 1) * 512],
                                     fo[2 * tt + cb][:],
                                     x2_t[tt][:, cb * 512:(cb + 1) * 512])
            nc.sync.dma_start(out_d[tt * 128:(tt + 1) * 128, :], o[:])

    xres.release()
    big.release()


def build_nc():
    nc = bacc.Bacc("TRN2", target_bir_lowering=False, debug=False,
                   num_devices=N_CORES)
    with tile.TileContext(nc) as tc:
        _body(tc)
    nc.compile()
    return nc


def _prep_weights(Wq, Wk, Wv, Wp, bp, W1, b1, W2, b2, g1, be1, g2, be2):
    bf = ml_dtypes.bfloat16
    g1 = g1.astype(np.float32)
    g2 = g2.astype(np.float32)

    def fold(W, g):
        return (g[:, None] * W.astype(np.float32))

    Wq_f, Wk_f, Wv_f = fold(Wq, g1), fold(Wk, g1), fold(Wv, g1)
    W1_f = fold(W1, g2)
    bq = be1.astype(np.float32) @ Wq.astype(np.float32)
    bk = be1.astype(np.float32) @ Wk.astype(np.float32)
    bv = be1.astype(np.float32) @ Wv.astype(np.float32)
    b1f = be2.astype(np.float32) @ W1.astype(np.float32) + b1.astype(np.float32)

    def tile_dt_c_kt(W, nblk):  # [C, N] -> [nblk, 128 c, C//128 kt, 128]
        kk = W.shape[0] // 128
        return np.ascontiguousarray(
            W.reshape(kk, 128, nblk, 128).transpose(2, 1, 0, 3)).astype(bf)

    wq_t = tile_dt_c_kt(Wq_f, 8)
    wk_t = tile_dt_c_kt(Wk_f, 8)
    wv_t = np.ascontiguousarray(Wv_f.reshape(8, 128, 1024)).astype(bf)
    wp_t = np.ascontiguousarray(
        Wp.astype(np.float32).reshape(8, 128, 1024)).astype(bf)
    w1_t = tile_dt_c_kt(W1_f, 32)
    w2_t = np.ascontiguousarray(
        W2.astype(np.float32).reshape(32, 128, 1024)).astype(bf)
    ident = np.eye(128).astype(bf)
    bcolq = np.ascontiguousarray(bq.reshape(8, 128).T).astype(np.float32)
    bcolk = np.ascontiguousarray(bk.reshape(8, 128).T).astype(np.float32)
    b1col = np.ascontiguousarray(b1f.reshape(32, 128).T).astype(np.float32)
    return dict(wq=wq_t, wk=wk_t, wv=wv_t, wp=wp_t, w1=w1_t, w2=w2_t,
                ident=ident, bcolq=bcolq, bcolk=bcolk, b1col=b1col,
                bv=bv.reshape(1, 1024).astype(bf),
                bp_r=bp.astype(np.float32).reshape(1, 1024).astype(bf),
                b2_r=b2.astype(np.float32).reshape(1, 1024).astype(bf))


def kernel(x, Wq, Wk, Wv, Wp, bp, W1, b1, W2, b2, g1, be1, g2, be2):
    global _CACHED_NC
    x = np.asarray(x, dtype=np.float32)
    if _CACHED_NC is None:
        _CACHED_NC = build_nc()
    nc = _CACHED_NC
    w = _prep_weights(np.asarray(Wq), np.asarray(Wk), np.asarray(Wv),
                      np.asarray(Wp), np.asarray(bp), np.asarray(W1),
                      np.asarray(b1), np.asarray(W2), np.asarray(b2),
                      np.asarray(g1), np.asarray(be1), np.asarray(g2),
                      np.asarray(be2))
    in_maps = []
    for c in range(N_CORES):
        b, q = c // 4, c % 4
        m = dict(w)
        m["x_own"] = np.ascontiguousarray(x[b, q * TOWN:(q + 1) * TOWN, :])
        m["x_kv"] = np.ascontiguousarray(x[b])
        in_maps.append(m)
    res = bass_utils.run_bass_kernel_spmd(nc, in_maps,
                                          core_ids=list(range(N_CORES)))
    out = np.empty((B, T, C), dtype=np.float32)
    for c in range(N_CORES):
        b, q = c // 4, c % 4
        out[b, q * TOWN:(q + 1) * TOWN, :] = res.results[c]["out"]
    return out
